# revision 14
# baseline (speedup 1.0000x reference)
"""Trainium2 Bass kernel for nn_BPDecoder: logits = 1 - exp(-exp(sum_i R_i*||Z_i||^2)).

Strategy (8-core SPMD, row-sharded, fp8 everywhere on device):
  - Pad N=500000 rows to 8 * 63488; core k takes rows [k*63488, (k+1)*63488).
  - Host scales Z by 512, casts to fp8 e4m3, and PRE-PERMUTES it into the
    exact on-device SBUF layout (slab-major: within a DMA slab of m tiles,
    partition p owns m*16 consecutive rows).  Every DMA is then a plain
    [128, m*2KB] 2D pattern with m*2KB contiguous per partition (8KB runs
    for the steady-state 4-tile slabs) -> HBM line rate.
  - Squares (fp8 -> fp8) are spread over THREE engines: ACT (~1.99us/tile),
    DVE (~2.29us/tile) and GPSIMD (~4.1us/tile), ratio 13/11/7.
  - R is cast to fp8 e4m3 on host (adds ~1e-3 rel err, gate is 2e-2) so the
    per-tile weighted reduction can use DoubleRow fp8 matmuls: per tile only
    2 matmuls [128,2,4]x[128,2,512] -> [4,512], PSUM-accumulated across all
    31 tiles into 2 banks.  Host extracts the q'==q diagonal blocks.
  - Final scalar: host sums diagonals, /512^2, applies 1 - exp(-exp(s)).
"""

import sys

sys.path.insert(0, "/opt/trn_rl_repo")


# The agent image lacks antenv.axon_hooks; recreate it so trace=True works
# (bass_utils imports it lazily for NTFF profiling under axon).
def _install_ntff_hook_shim():
    import types
    if "antenv.axon_hooks" in sys.modules:
        return
    mod = types.ModuleType("antenv.axon_hooks")
    state = {"hook": None}
    mod.set_axon_ntff_profile_hook = lambda h: state.__setitem__("hook", h)
    mod.get_axon_ntff_profile_hook = lambda: state["hook"]
    sys.modules["antenv.axon_hooks"] = mod
    try:
        sys.path.insert(0, "/root/.axon_site")
        from trn_agent_boot.trn_boot import _ntff_profile_via_ctypes
        state["hook"] = _ntff_profile_via_ctypes("/opt/axon/libaxon_pjrt.so")
    except Exception:
        pass


_install_ntff_hook_shim()

import numpy as np

import concourse.bass as bass
import concourse.bacc as bacc
import concourse.mybir as mybir
from concourse.tile import TileContext
from concourse.bass_utils import run_bass_kernel_spmd

P = 128          # SBUF partitions
D = 128          # row length (feature dim)
Q = 16           # rows per partition per tile
FREE = Q * D     # free elems per tile = 2048
T = 31           # tiles per core
NC_ROWS = T * P * Q   # 63488 rows per core
N_CORES = 8
N_FULL = 500000

Z_DT = mybir.dt.float8e4
R_DT = mybir.dt.float8e4    # DoubleRow matmul needs fp8 stationary
RB_DT = mybir.dt.bfloat16   # stationary for the bf16-moving matmuls
S_DT = mybir.dt.float8e4    # squared ACT tile (DoubleRow rhs)
SB_DT = mybir.dt.bfloat16   # squared DVE/GPS tile (bf16 rhs)

Z_SCALE_IN = 512.0         # host multiplies Z by this before the fp8 cast

# DMA slabs (tiles per dma_start); small head slabs shorten the ramp
SLAB_SIZES = [1, 2, 4, 4, 4, 4, 4, 4, 4]
assert sum(SLAB_SIZES) == T
SLAB_MAX = max(SLAB_SIZES)

# square-engine split per slab; "act" tiles square fp8->fp8 (DoubleRow mms),
# "dve"/"gps" tiles square fp8->bf16 (bf16 mms; DVE fp8-out measures 4.1us,
# bf16-out 2.3us).  Adjacent ACT tiles in a slab are fused into one
# instruction to amortize the ~0.3us ACT instruction overhead.
SLAB_PATTERNS = [
    ["dve"],
    ["act", "act"],
    ["act", "act", "dve", "gps"],
    ["act", "act", "dve", "gps"],
    ["act", "act", "dve", "gps"],
    ["act", "act", "dve", "gps"],
    ["act", "dve", "dve", "gps"],
    ["act", "dve", "dve", "gps"],
    ["act", "dve", "dve", "dve"],
]
SQ_PATTERN = [e for pat in SLAB_PATTERNS for e in pat]
assert [len(p) for p in SLAB_PATTERNS] == SLAB_SIZES
N_ACT = SQ_PATTERN.count("act")
N_DVE = SQ_PATTERN.count("dve")
N_GPS = SQ_PATTERN.count("gps")
ACT_TILES = [i for i, e in enumerate(SQ_PATTERN) if e == "act"]
BF_TILES = [i for i, e in enumerate(SQ_PATTERN) if e != "act"]

_cache = {}


def _np_dt(dt):
    return mybir.dt.np(dt)


def _build():
    nc = bacc.Bacc(trn_type="TRN2")
    # host pre-permutes into the exact on-device layouts
    z = nc.declare_dram_parameter("z", [P, T, 4, 512], Z_DT, isOutput=False)
    r = nc.declare_dram_parameter("r", [P, T, 4, 32], R_DT, isOutput=False)
    rb = nc.declare_dram_parameter("rb", [P, T * Q], RB_DT, isOutput=False)
    out = nc.declare_dram_parameter("out", [4, 1024], mybir.dt.float32, isOutput=True)
    outb = nc.declare_dram_parameter("outb", [Q, FREE], mybir.dt.float32, isOutput=True)

    slabs = []
    pos = 0
    for sz in SLAB_SIZES:
        slabs.append((pos, pos + sz))
        pos += sz
    act_first, act_last = ACT_TILES[0], ACT_TILES[-1]
    bf_first, bf_last = BF_TILES[0], BF_TILES[-1]

    with TileContext(nc) as tc:
        with (
            tc.tile_pool(name="zpool", bufs=3) as zpool,
            tc.tile_pool(name="spool", bufs=3) as spool,
            tc.tile_pool(name="singles", bufs=1) as singles,
            tc.tile_pool(name="ppool", bufs=1, space="PSUM") as ppool,
        ):
            r_sb = singles.tile([P, T, 4, 32], R_DT)
            nc.scalar.dma_start(out=r_sb[:], in_=r[:])
            rb_sb = singles.tile([P, T * Q], RB_DT)
            nc.scalar.dma_start(out=rb_sb[:], in_=rb[:])

            # DoubleRow accumulators (stationary padded to 32 cols: R in cols
            # 0-3, zeros after -- ldweights wants a full 32-wide column tile)
            accs = [ppool.tile([32, 512], mybir.dt.float32, name=f"acc{h}")
                    for h in range(2)]
            # bf16-matmul accumulator, 4 PSUM banks; slice sl holds q'=4sl..4sl+3
            accb = ppool.tile([Q, FREE], mybir.dt.float32, name="accb")

            for si, (t0, t1) in enumerate(slabs):
                m = t1 - t0
                z_sb = zpool.tile([P, SLAB_MAX, 4, 512], Z_DT, tag="z")
                nc.sync.dma_start(out=z_sb[:, :m], in_=z[:, t0:t1])
            # all z slabs stream on the (otherwise idle) sync HWDGE ring
                s_sb = spool.tile([P, SLAB_MAX, 4, 512], S_DT, tag="s")
                sb_sb = spool.tile([P, SLAB_MAX, 4, 512], SB_DT, tag="sb")
                t = t0
                while t < t1:
                    ti = t - t0
                    se = SQ_PATTERN[t]
                    if se == "act":
                        # fuse adjacent ACT tiles into one instruction
                        n = 1
                        if t + 1 < t1 and SQ_PATTERN[t + 1] == "act":
                            n = 2
                        nc.scalar.square(s_sb[:, ti:ti + n], z_sb[:, ti:ti + n])
                        for tt in range(t, t + n):
                            tti = tt - t0
                            for h in range(2):
                                nc.tensor.matmul(
                                    accs[h][:],
                                    r_sb[:, tt, 2 * h:2 * h + 2, :],
                                    s_sb[:, tti, 2 * h:2 * h + 2, :],
                                    start=(tt == act_first),
                                    stop=(tt == act_last),
                                    perf_mode=mybir.MatmulPerfMode.DoubleRow,
                                )
                        t += n
                        continue
                    if se == "dve":
                        nc.vector.tensor_mul(sb_sb[:, ti], z_sb[:, ti], z_sb[:, ti])
                    else:
                        nc.gpsimd.tensor_mul(sb_sb[:, ti], z_sb[:, ti], z_sb[:, ti])
                    for sl in range(4):
                        nc.tensor.matmul(
                            accb[:, sl * 512:(sl + 1) * 512],
                            rb_sb[:, t * Q:(t + 1) * Q],
                            sb_sb[:, ti, sl, :],
                            start=(t == bf_first),
                            stop=(t == bf_last),
                        )
                    t += 1

            out_sb = singles.tile([4, 1024], mybir.dt.float32)
            nc.vector.tensor_copy(out_sb[:, 0:512], accs[0][0:4, :])
            nc.vector.tensor_copy(out_sb[:, 512:1024], accs[1][0:4, :])
            nc.scalar.dma_start(out=out[:], in_=out_sb[:])
            outb_sb = singles.tile([Q, FREE], mybir.dt.float32)
            for sl in range(4):
                copy_eng = nc.scalar.copy if sl % 2 == 0 else nc.vector.tensor_copy
                copy_eng(outb_sb[:, sl * 512:(sl + 1) * 512],
                         accb[:, sl * 512:(sl + 1) * 512])
            nc.sync.dma_start(out=outb[:], in_=outb_sb[:])
    nc.compile()
    return nc


def _get_nc():
    if "nc" not in _cache:
        _cache["nc"] = _build()
    return _cache["nc"]


def _shard(Z, R):
    np_z = _np_dt(Z_DT)
    np_r = _np_dt(R_DT)
    ZP = np.zeros((N_CORES * NC_ROWS, D), dtype=np_z)
    ZP[:N_FULL] = (Z * np.float32(Z_SCALE_IN)).astype(np_z)
    RP = np.zeros((N_CORES * NC_ROWS,), dtype=np_r)
    RP[:N_FULL] = R.astype(np_r, copy=False)
    ZP = ZP.reshape(N_CORES, NC_ROWS, D)
    RP = RP.reshape(N_CORES, NC_ROWS)

    # slab-major permutation: within slab (t0, m), partition p owns rows
    # [t0*2048 + p*m*16, +m*16); device column for (t, q, d) is t*2048+q*128+d
    ZD = np.empty((N_CORES, P, T * FREE), dtype=np_z)
    RD = np.empty((N_CORES, P, T * Q), dtype=np_r)
    pos = 0
    for m in SLAB_SIZES:
        t0 = pos
        zb = ZP[:, t0 * 2048:(t0 + m) * 2048].reshape(N_CORES, P, m * Q, D)
        ZD[:, :, t0 * FREE:(t0 + m) * FREE] = zb.reshape(N_CORES, P, m * FREE)
        rb = RP[:, t0 * 2048:(t0 + m) * 2048].reshape(N_CORES, P, m * Q)
        RD[:, :, t0 * Q:(t0 + m) * Q] = rb
        pos += m
    ZD = ZD.reshape(N_CORES, P, T, 4, 512)
    RDf = RD.reshape(N_CORES, P, T, 4, 4)
    RD32 = np.zeros((N_CORES, P, T, 4, 32), dtype=np_r)
    RD32[..., 0:4] = RDf
    RB = RD.astype(_np_dt(RB_DT))
    return [{"z": ZD[k], "r": RD32[k], "rb": RB[k]} for k in range(N_CORES)]


def _combine(results):
    s = 0.0
    idx = np.arange(4)
    idq = np.arange(Q)
    for res in results:
        # out [4, 1024] -> [m, h, qq, d]; diagonal blocks are qq == m
        C = np.asarray(res["out"], dtype=np.float64).reshape(4, 2, 4, D)
        s += C[idx, :, idx, :].sum()
        # outb [16, 2048] -> [q', q, d]; diagonal blocks are q' == q
        Cb = np.asarray(res["outb"], dtype=np.float64).reshape(Q, Q, D)
        s += Cb[idq, idq, :].sum()
    s /= float(Z_SCALE_IN) ** 2
    lam = np.exp(s)
    logits = 1.0 - np.exp(-lam)
    return np.float32(logits)


def _run(Z, R, trace=False, tmpdir=None):
    nc = _get_nc()
    in_maps = _shard(Z, R)
    return run_bass_kernel_spmd(nc, in_maps, core_ids=list(range(N_CORES)),
                                trace=trace, tmpdir=tmpdir)


def kernel(Z, R):
    assert Z.shape == (N_FULL, D) and R.shape == (N_FULL,)
    out = _run(np.asarray(Z), np.asarray(R), trace=False)
    return _combine(out.results)


# revision 15
# speedup vs baseline: 1.2158x; 1.2158x over previous
"""Trainium2 Bass kernel for nn_BPDecoder: logits = 1 - exp(-exp(sum_i R_i*||Z_i||^2)).

Strategy (8-core SPMD, row-sharded, fp8 everywhere on device):
  - Pad N=500000 rows to 8 * 63488; core k takes rows [k*63488, (k+1)*63488).
  - Host scales Z by 512, casts to fp8 e4m3, and PRE-PERMUTES it into the
    exact on-device SBUF layout (slab-major: within a DMA slab of m tiles,
    partition p owns m*16 consecutive rows).  Every DMA is then a plain
    [128, m*2KB] 2D pattern with m*2KB contiguous per partition (8KB runs
    for the steady-state 4-tile slabs) -> HBM line rate.
  - Squares (fp8 -> fp8) are spread over THREE engines: ACT (~1.99us/tile),
    DVE (~2.29us/tile) and GPSIMD (~4.1us/tile), ratio 13/11/7.
  - R is cast to fp8 e4m3 on host (adds ~1e-3 rel err, gate is 2e-2) so the
    per-tile weighted reduction can use DoubleRow fp8 matmuls: per tile only
    2 matmuls [128,2,4]x[128,2,512] -> [4,512], PSUM-accumulated across all
    31 tiles into 2 banks.  Host extracts the q'==q diagonal blocks.
  - Final scalar: host sums diagonals, /512^2, applies 1 - exp(-exp(s)).
"""

import sys

sys.path.insert(0, "/opt/trn_rl_repo")


# The agent image lacks antenv.axon_hooks; recreate it so trace=True works
# (bass_utils imports it lazily for NTFF profiling under axon).
def _install_ntff_hook_shim():
    import types
    if "antenv.axon_hooks" in sys.modules:
        return
    mod = types.ModuleType("antenv.axon_hooks")
    state = {"hook": None}
    mod.set_axon_ntff_profile_hook = lambda h: state.__setitem__("hook", h)
    mod.get_axon_ntff_profile_hook = lambda: state["hook"]
    sys.modules["antenv.axon_hooks"] = mod
    try:
        sys.path.insert(0, "/root/.axon_site")
        from trn_agent_boot.trn_boot import _ntff_profile_via_ctypes
        state["hook"] = _ntff_profile_via_ctypes("/opt/axon/libaxon_pjrt.so")
    except Exception:
        pass


_install_ntff_hook_shim()

import numpy as np

import concourse.bass as bass
import concourse.bacc as bacc
import concourse.mybir as mybir
from concourse.tile import TileContext
from concourse.bass_utils import run_bass_kernel_spmd

P = 128          # SBUF partitions
D = 128          # row length (feature dim)
Q = 16           # rows per partition per tile
FREE = Q * D     # free elems per tile = 2048
T = 31           # tiles per core
NC_ROWS = T * P * Q   # 63488 rows per core
N_CORES = 8
N_FULL = 500000

Z_DT = mybir.dt.float8e4
R_DT = mybir.dt.float8e4    # DoubleRow matmul needs fp8 stationary
RB_DT = mybir.dt.bfloat16   # stationary for the bf16-moving matmuls
S_DT = mybir.dt.float8e4    # squared ACT tile (DoubleRow rhs)
SB_DT = mybir.dt.bfloat16   # squared DVE/GPS tile (bf16 rhs)

Z_SCALE_IN = 512.0         # host multiplies Z by this before the fp8 cast

# DMA slabs (tiles per dma_start); small head slabs shorten the ramp
SLAB_SIZES = [1, 2, 4, 4, 4, 4, 4, 4, 4]
assert sum(SLAB_SIZES) == T
SLAB_MAX = max(SLAB_SIZES)

# square-engine split per slab; "act" tiles square fp8->fp8 (DoubleRow mms),
# "dve"/"gps" tiles square fp8->bf16 (bf16 mms; DVE fp8-out measures 4.1us,
# bf16-out 2.3us).  Adjacent ACT tiles in a slab are fused into one
# instruction to amortize the ~0.3us ACT instruction overhead.
# NOTE: no GPSIMD tiles -- GpSimd tensor ops contend with DVE for the shared
# SBUF port pair and slow DVE squares from 2.3us to 3.9us/tile (measured).
SLAB_PATTERNS = [
    ["dve"],
    ["act", "act"],
    ["act", "act", "dve", "dve"],
    ["act", "act", "dve", "dve"],
    ["act", "act", "dve", "dve"],
    ["act", "act", "dve", "dve"],
    ["act", "act", "dve", "dve"],
    ["act", "act", "dve", "dve"],
    ["act", "act", "dve", "dve"],
]
SQ_PATTERN = [e for pat in SLAB_PATTERNS for e in pat]
assert [len(p) for p in SLAB_PATTERNS] == SLAB_SIZES
N_ACT = SQ_PATTERN.count("act")
N_DVE = SQ_PATTERN.count("dve")
N_GPS = SQ_PATTERN.count("gps")
ACT_TILES = [i for i, e in enumerate(SQ_PATTERN) if e == "act"]
BF_TILES = [i for i, e in enumerate(SQ_PATTERN) if e != "act"]

_cache = {}


def _np_dt(dt):
    return mybir.dt.np(dt)


def _build():
    nc = bacc.Bacc(trn_type="TRN2")
    # host pre-permutes into the exact on-device layouts
    z = nc.declare_dram_parameter("z", [P, T, 4, 512], Z_DT, isOutput=False)
    r = nc.declare_dram_parameter("r", [P, T, 4, 32], R_DT, isOutput=False)
    rb = nc.declare_dram_parameter("rb", [P, T * Q], RB_DT, isOutput=False)
    out = nc.declare_dram_parameter("out", [4, 1024], mybir.dt.float32, isOutput=True)
    outb = nc.declare_dram_parameter("outb", [Q, FREE], mybir.dt.float32, isOutput=True)

    slabs = []
    pos = 0
    for sz in SLAB_SIZES:
        slabs.append((pos, pos + sz))
        pos += sz
    act_first, act_last = ACT_TILES[0], ACT_TILES[-1]
    bf_first, bf_last = BF_TILES[0], BF_TILES[-1]

    with TileContext(nc) as tc:
        with (
            tc.tile_pool(name="zpool", bufs=3) as zpool,
            tc.tile_pool(name="spool", bufs=3) as spool,
            tc.tile_pool(name="singles", bufs=1) as singles,
            tc.tile_pool(name="ppool", bufs=1, space="PSUM") as ppool,
        ):
            r_sb = singles.tile([P, T, 4, 32], R_DT)
            nc.scalar.dma_start(out=r_sb[:], in_=r[:])
            rb_sb = singles.tile([P, T * Q], RB_DT)
            nc.scalar.dma_start(out=rb_sb[:], in_=rb[:])
            # tiny dummy square: loads the ACT Square table (~1.3us) during
            # the first z-slab's DMA instead of on the critical path
            warm_sb = singles.tile([P, 2], mybir.dt.bfloat16)
            nc.scalar.square(warm_sb[:, 0:1], warm_sb[:, 1:2])

            # DoubleRow accumulators (stationary padded to 32 cols: R in cols
            # 0-3, zeros after -- ldweights wants a full 32-wide column tile)
            accs = [ppool.tile([32, 512], mybir.dt.float32, name=f"acc{h}")
                    for h in range(2)]
            # bf16-matmul accumulator, 4 PSUM banks; slice sl holds q'=4sl..4sl+3
            accb = ppool.tile([Q, FREE], mybir.dt.float32, name="accb")

            for si, (t0, t1) in enumerate(slabs):
                m = t1 - t0
                z_sb = zpool.tile([P, SLAB_MAX, 4, 512], Z_DT, tag="z")
                nc.sync.dma_start(out=z_sb[:, :m], in_=z[:, t0:t1])
            # all z slabs stream on the (otherwise idle) sync HWDGE ring
                s_sb = spool.tile([P, SLAB_MAX, 4, 512], S_DT, tag="s")
                sb_sb = spool.tile([P, SLAB_MAX, 4, 512], SB_DT, tag="sb")
                t = t0
                while t < t1:
                    ti = t - t0
                    se = SQ_PATTERN[t]
                    if se == "act":
                        # fuse adjacent ACT tiles into one instruction
                        n = 1
                        if t + 1 < t1 and SQ_PATTERN[t + 1] == "act":
                            n = 2
                        nc.scalar.square(s_sb[:, ti:ti + n], z_sb[:, ti:ti + n])
                        for tt in range(t, t + n):
                            tti = tt - t0
                            for h in range(2):
                                nc.tensor.matmul(
                                    accs[h][:],
                                    r_sb[:, tt, 2 * h:2 * h + 2, :],
                                    s_sb[:, tti, 2 * h:2 * h + 2, :],
                                    start=(tt == act_first),
                                    stop=(tt == act_last),
                                    perf_mode=mybir.MatmulPerfMode.DoubleRow,
                                )
                        t += n
                        continue
                    if se == "dve":
                        nc.vector.tensor_mul(sb_sb[:, ti], z_sb[:, ti], z_sb[:, ti])
                    else:
                        nc.gpsimd.tensor_mul(sb_sb[:, ti], z_sb[:, ti], z_sb[:, ti])
                    for sl in range(4):
                        nc.tensor.matmul(
                            accb[:, sl * 512:(sl + 1) * 512],
                            rb_sb[:, t * Q:(t + 1) * Q],
                            sb_sb[:, ti, sl, :],
                            start=(t == bf_first),
                            stop=(t == bf_last),
                        )
                    t += 1

            out_sb = singles.tile([4, 1024], mybir.dt.float32)
            nc.scalar.copy(out_sb[:, 0:512], accs[0][0:4, :])
            nc.scalar.copy(out_sb[:, 512:1024], accs[1][0:4, :])
            nc.scalar.dma_start(out=out[:], in_=out_sb[:])
            outb_sb = singles.tile([Q, FREE], mybir.dt.float32)
            for sl in range(4):
                copy_eng = nc.scalar.copy if sl % 2 == 0 else nc.vector.tensor_copy
                copy_eng(outb_sb[:, sl * 512:(sl + 1) * 512],
                         accb[:, sl * 512:(sl + 1) * 512])
            nc.sync.dma_start(out=outb[:], in_=outb_sb[:])
    nc.compile()
    return nc


def _get_nc():
    if "nc" not in _cache:
        _cache["nc"] = _build()
    return _cache["nc"]


def _shard(Z, R):
    np_z = _np_dt(Z_DT)
    np_r = _np_dt(R_DT)
    ZP = np.zeros((N_CORES * NC_ROWS, D), dtype=np_z)
    ZP[:N_FULL] = (Z * np.float32(Z_SCALE_IN)).astype(np_z)
    RP = np.zeros((N_CORES * NC_ROWS,), dtype=np_r)
    RP[:N_FULL] = R.astype(np_r, copy=False)
    ZP = ZP.reshape(N_CORES, NC_ROWS, D)
    RP = RP.reshape(N_CORES, NC_ROWS)

    # slab-major permutation: within slab (t0, m), partition p owns rows
    # [t0*2048 + p*m*16, +m*16); device column for (t, q, d) is t*2048+q*128+d
    ZD = np.empty((N_CORES, P, T * FREE), dtype=np_z)
    RD = np.empty((N_CORES, P, T * Q), dtype=np_r)
    pos = 0
    for m in SLAB_SIZES:
        t0 = pos
        zb = ZP[:, t0 * 2048:(t0 + m) * 2048].reshape(N_CORES, P, m * Q, D)
        ZD[:, :, t0 * FREE:(t0 + m) * FREE] = zb.reshape(N_CORES, P, m * FREE)
        rb = RP[:, t0 * 2048:(t0 + m) * 2048].reshape(N_CORES, P, m * Q)
        RD[:, :, t0 * Q:(t0 + m) * Q] = rb
        pos += m
    ZD = ZD.reshape(N_CORES, P, T, 4, 512)
    RDf = RD.reshape(N_CORES, P, T, 4, 4)
    RD32 = np.zeros((N_CORES, P, T, 4, 32), dtype=np_r)
    RD32[..., 0:4] = RDf
    RB = RD.astype(_np_dt(RB_DT))
    return [{"z": ZD[k], "r": RD32[k], "rb": RB[k]} for k in range(N_CORES)]


def _combine(results):
    s = 0.0
    idx = np.arange(4)
    idq = np.arange(Q)
    for res in results:
        # out [4, 1024] -> [m, h, qq, d]; diagonal blocks are qq == m
        C = np.asarray(res["out"], dtype=np.float64).reshape(4, 2, 4, D)
        s += C[idx, :, idx, :].sum()
        # outb [16, 2048] -> [q', q, d]; diagonal blocks are q' == q
        Cb = np.asarray(res["outb"], dtype=np.float64).reshape(Q, Q, D)
        s += Cb[idq, idq, :].sum()
    s /= float(Z_SCALE_IN) ** 2
    lam = np.exp(s)
    logits = 1.0 - np.exp(-lam)
    return np.float32(logits)


def _run(Z, R, trace=False, tmpdir=None):
    nc = _get_nc()
    in_maps = _shard(Z, R)
    return run_bass_kernel_spmd(nc, in_maps, core_ids=list(range(N_CORES)),
                                trace=trace, tmpdir=tmpdir)


def kernel(Z, R):
    assert Z.shape == (N_FULL, D) and R.shape == (N_FULL,)
    out = _run(np.asarray(Z), np.asarray(R), trace=False)
    return _combine(out.results)
